# revision 1
# baseline (speedup 1.0000x reference)
"""Bilateral grid slice+apply on 8 Trainium2 NeuronCores.

Gather-free formulation: the per-pixel trilinear interpolation is expressed
in the hat-function basis  hat(a) = relu(1 - |a|)  and evaluated densely as
matmuls with the (tiny) grid as the stationary operand:

    coeffs[n, z, c] = sum_{y,x} hy(n,y) hx(n,x) * G[y, x, z, c]     (PE, K=256)
    out[n, c3]      = sum_{z,j} hz(n,z) * xt[n,j] * coeffs[n, z, 4c3+j]

Pixels ride the matmul free dimension (512 per tile); hats are built with one
PE broadcast matmul + Abs/Relu activations; the z/affine fold is two
elementwise muls + a final K=96 reduce matmul.

Data parallel: pixels are sharded across the 8 cores; the 16x16x8x12 grid is
replicated (host bakes it into the stationary operands).
"""
import numpy as np
from contextlib import ExitStack

import concourse.bass as bass
import concourse.bacc as bacc
import concourse.mybir as mybir
from concourse import tile
from concourse.bass_utils import run_bass_kernel_spmd

F = 512             # pixels per tile (one fp32 PSUM bank)
NCORES = 8
B, H, W = 4, 1080, 1920
NTOT = B * H * W                  # 8294400
NPC = NTOT // NCORES              # 1036800 per core
T = NPC // F                      # 2025 tiles per core
LUM = (0.2126, 0.7152, 0.0722)

_CACHE = {}


def _make_stationaries(grid):
    g = grid.astype(np.float32)
    stP0 = np.zeros((5, 72), np.float32)      # rows (r,g,b,cx,cy)
    for m in range(16):
        stP0[4, m] = 15.0                     # gy from cy
    for m in range(32, 48):
        stP0[3, m] = 15.0                     # gx from cx
    for m in range(64, 72):
        stP0[0, m] = 7.0 * LUM[0]
        stP0[1, m] = 7.0 * LUM[1]
        stP0[2, m] = 7.0 * LUM[2]
    bias40 = np.zeros((72, 1), np.float32)
    bias40[:16, 0] = -np.arange(16)
    bias40[32:48, 0] = -np.arange(16)
    bias40[64:72, 0] = -np.arange(8)

    stHY = np.zeros((2, 16, 128), np.float32)
    stHX = np.zeros((2, 16, 128), np.float32)
    for p in range(2):
        for m in range(128):
            stHY[p, p * 8 + m // 16, m] = 1.0
            stHX[p, m % 16, m] = 1.0

    stMAIN = np.zeros((2, 128, 96), np.float32)
    for p in range(2):
        for k in range(128):
            stMAIN[p, k, :] = g[p * 8 + k // 16, k % 16].reshape(96)

    stHZ = np.zeros((8, 96), np.float32)
    for z in range(8):
        stHZ[z, z * 12:(z + 1) * 12] = 1.0

    stX = np.zeros((4, 96), np.float32)       # rhs rows (ones, r, g, b)
    for z in range(8):
        for c3 in range(3):
            for j in range(4):
                stX[0 if j == 3 else j + 1, z * 12 + c3 * 4 + j] = 1.0

    stRED = np.zeros((96, 3), np.float32)
    for z in range(8):
        for c3 in range(3):
            for j in range(4):
                stRED[z * 12 + c3 * 4 + j, c3] = 1.0

    return dict(stP0=stP0, bias40=bias40,
                stHYa=stHY[0], stHYb=stHY[1], stHXa=stHX[0], stHXb=stHX[1],
                stMAINa=stMAIN[0], stMAINb=stMAIN[1],
                stHZ=stHZ, stX=stX, stRED=stRED)


def build_kernel(ntiles=T, num_cores=NCORES, reps=1):
    nc = bacc.Bacc("TRN2", target_bir_lowering=False, debug=False,
                   num_devices=num_cores)
    NP = ntiles * F
    f32 = mybir.dt.float32

    in5 = nc.declare_dram_parameter("in5", [5, NP], f32, isOutput=False)
    inx = nc.declare_dram_parameter("inx", [4, NP], f32, isOutput=False)
    decls = {}
    for nm, shp in (("stP0", [5, 72]), ("bias40", [72, 1]),
                    ("stHYa", [16, 128]), ("stHYb", [16, 128]),
                    ("stHXa", [16, 128]), ("stHXb", [16, 128]),
                    ("stMAINa", [128, 96]), ("stMAINb", [128, 96]),
                    ("stHZ", [8, 96]), ("stX", [4, 96]), ("stRED", [96, 3])):
        decls[nm] = nc.declare_dram_parameter(nm, shp, f32, isOutput=False)
    out3 = nc.declare_dram_parameter("out3", [3, NP], f32, isOutput=True)

    with tile.TileContext(nc) as tc:
        with ExitStack() as ctx:
            stp = ctx.enter_context(tc.tile_pool(name="stats", bufs=1))
            sP0 = stp.tile([5, 72], f32, tag="sP0")
            sB40 = stp.tile([72, 1], f32, tag="sB40")
            sHYa = stp.tile([16, 128], f32, tag="sHYa")
            sHYb = stp.tile([16, 128], f32, tag="sHYb")
            sHXa_t = stp.tile([48, 128], f32, tag="sHXa")
            sHXb_t = stp.tile([48, 128], f32, tag="sHXb")
            sHXa = sHXa_t[32:48, :]
            sHXb = sHXb_t[32:48, :]
            sMa = stp.tile([128, 96], f32, tag="sMa")
            sMb = stp.tile([128, 96], f32, tag="sMb")
            sHZ_t = stp.tile([72, 96], f32, tag="sHZ")
            sX_t = stp.tile([36, 96], f32, tag="sX")
            sHZ = sHZ_t[64:72, :]
            sX = sX_t[32:36, :]
            sRED = stp.tile([96, 3], f32, tag="sRED")
            for t_, nm in ((sP0[:], "stP0"), (sB40[:], "bias40"),
                           (sHYa[:], "stHYa"), (sHYb[:], "stHYb"),
                           (sHXa, "stHXa"), (sHXb, "stHXb"),
                           (sMa[:], "stMAINa"), (sMb[:], "stMAINb"),
                           (sHZ, "stHZ"), (sX, "stX"), (sRED[:], "stRED")):
                nc.sync.dma_start(t_, decls[nm].ap())

            sb_in = ctx.enter_context(tc.tile_pool(name="sb_in", bufs=3))
            sb_mid = ctx.enter_context(tc.tile_pool(name="sb_mid", bufs=3))
            sb_w = ctx.enter_context(tc.tile_pool(name="sb_w", bufs=2))
            sb_wab = ctx.enter_context(tc.tile_pool(name="sb_wab", bufs=1))
            ps_args = ctx.enter_context(tc.tile_pool(name="ps_args", bufs=1, space="PSUM"))
            ps_rep = ctx.enter_context(tc.tile_pool(name="ps_rep", bufs=2, space="PSUM"))
            ps_rep2 = ctx.enter_context(tc.tile_pool(name="ps_rep2", bufs=2, space="PSUM"))
            ps_cf = ctx.enter_context(tc.tile_pool(name="ps_cf", bufs=1, space="PSUM"))
            ps_zx = ctx.enter_context(tc.tile_pool(name="ps_zx", bufs=1, space="PSUM"))

            G = 6
            for _rep in range(reps):
              for g0 in range(0, ntiles, G):
                gtiles = range(g0, min(g0 + G, ntiles))
                ins, hats_l, Wa_l, Wb_l = {}, {}, {}, {}
                for i in gtiles:      # phase 1: load, hat args, hats
                    IN6 = sb_in.tile([36, F], f32, tag=f"in6_{i%(G+1)}")
                    nc.sync.dma_start(IN6[0:5, :], in5.ap()[:, bass.ts(i, F)])
                    nc.sync.dma_start(IN6[32:36, :], inx.ap()[:, bass.ts(i, F)])
                    ins[i] = IN6
                    argsP = ps_args.tile([72, F], f32, tag="args")
                    nc.tensor.matmul(argsP[:], sP0[:], IN6[0:5, :], start=True, stop=True)
                    tabs = sb_mid.tile([72, F], f32, tag="tabs")
                    nc.scalar.activation(tabs[:], argsP[:],
                                         mybir.ActivationFunctionType.Abs,
                                         bias=sB40[:], scale=1.0)
                    hats = sb_mid.tile([72, F], f32, tag=f"hats_{i%(G+1)}")
                    nc.scalar.activation(hats[:], tabs[:],
                                         mybir.ActivationFunctionType.Relu,
                                         bias=1.0, scale=-1.0)
                    hats_l[i] = hats
                for i in gtiles:      # phase 2: W = hy (x) hx
                    hats = hats_l[i]
                    HYa = ps_rep.tile([128, F], f32, tag="HY")
                    HXa = ps_rep2.tile([128, F], f32, tag="HX")
                    nc.tensor.matmul(HYa[:], sHYa[:], hats[0:16, :], start=True, stop=True)
                    nc.tensor.matmul(HXa[:], sHXa, hats[32:48, :], start=True, stop=True)
                    HXaS = sb_w.tile([128, F], f32, tag="HXaS")
                    nc.scalar.copy(HXaS[:], HXa[:])
                    Wa = sb_wab.tile([128, F], f32, tag=f"Wa_{i%(G+1)}")
                    nc.vector.tensor_tensor(out=Wa[:], in0=HYa[:], in1=HXaS[:],
                                            op=mybir.AluOpType.mult)
                    HYb = ps_rep.tile([128, F], f32, tag="HY")
                    HXb = ps_rep2.tile([128, F], f32, tag="HX")
                    nc.tensor.matmul(HYb[:], sHYb[:], hats[0:16, :], start=True, stop=True)
                    nc.tensor.matmul(HXb[:], sHXb, hats[32:48, :], start=True, stop=True)
                    HXbS = sb_w.tile([128, F], f32, tag="HXbS")
                    nc.vector.tensor_copy(HXbS[:], HXb[:])
                    Wb = sb_wab.tile([128, F], f32, tag=f"Wb_{i%(G+1)}")
                    nc.vector.tensor_tensor(out=Wb[:], in0=HYb[:], in1=HXbS[:],
                                            op=mybir.AluOpType.mult)
                    Wa_l[i], Wb_l[i] = Wa, Wb
                for i in gtiles:      # phase 3: mains + z/affine fold + out
                    hats, IN6 = hats_l[i], ins[i]
                    CF = ps_cf.tile([96, F], f32, tag="CF")
                    nc.tensor.matmul(CF[:], sMa[:], Wa_l[i][:], start=True, stop=False)
                    nc.tensor.matmul(CF[:], sMb[:], Wb_l[i][:], start=False, stop=True)
                    HZ96 = ps_zx.tile([96, F], f32, tag="HZ")
                    X96 = ps_cf.tile([96, F], f32, tag="X96")
                    nc.tensor.matmul(HZ96[:], sHZ, hats[64:72, :], start=True, stop=True)
                    nc.tensor.matmul(X96[:], sX, IN6[32:36, :], start=True, stop=True)
                    HZS = sb_w.tile([96, F], f32, tag="HZS")
                    nc.scalar.copy(HZS[:], HZ96[:])
                    HZX = sb_w.tile([96, F], f32, tag="HZX")
                    nc.vector.tensor_tensor(out=HZX[:], in0=X96[:], in1=HZS[:],
                                            op=mybir.AluOpType.mult)
                    M2 = sb_w.tile([96, F], f32, tag="M2")
                    nc.vector.tensor_tensor(out=M2[:], in0=CF[:], in1=HZX[:],
                                            op=mybir.AluOpType.mult)
                    OUT3 = ps_zx.tile([3, F], f32, tag="HZ")
                    nc.tensor.matmul(OUT3[:], sRED[:], M2[:], start=True, stop=True)
                    OUTS = sb_in.tile([3, F], f32, tag="outs")
                    nc.scalar.copy(OUTS[:], OUT3[:])
                    nc.sync.dma_start(out3.ap()[:, bass.ts(i, F)], OUTS[:])

    nc.compile()
    return nc


def kernel(pixels: np.ndarray, coords: np.ndarray, grid: np.ndarray) -> np.ndarray:
    assert pixels.shape == (B, H, W, 3) and coords.shape == (B, H, W, 2)
    p = np.asarray(pixels, np.float32).reshape(-1, 3)
    c = np.asarray(coords, np.float32).reshape(-1, 2)
    r = np.ascontiguousarray(p[:, 0]); g = np.ascontiguousarray(p[:, 1])
    b = np.ascontiguousarray(p[:, 2])
    cx = np.ascontiguousarray(c[:, 0]); cy = np.ascontiguousarray(c[:, 1])
    ones = np.ones(NPC, np.float32)

    stats = _make_stationaries(np.asarray(grid, np.float32))
    in_maps = []
    for cid in range(NCORES):
        s = slice(cid * NPC, (cid + 1) * NPC)
        in5 = np.ascontiguousarray(np.stack([r[s], g[s], b[s], cx[s], cy[s]]))
        inx = np.ascontiguousarray(np.stack([ones, r[s], g[s], b[s]]))
        in_maps.append({"in5": in5, "inx": inx, **stats})

    if "nc" not in _CACHE:
        _CACHE["nc"] = build_kernel()
    nc = _CACHE["nc"]
    res = run_bass_kernel_spmd(nc, in_maps, list(range(NCORES)))
    out = np.concatenate([res.results[cid]["out3"].T for cid in range(NCORES)], 0)
    return np.ascontiguousarray(out.reshape(B, H, W, 3).astype(np.float32))
